# revision 1
# baseline (speedup 1.0000x reference)
import jax
import jax.numpy as jnp
import numpy as np
import hashlib

# nn_ClassifierDeformable: 6 deformable-conv layers (fixed offsets shared
# across batch) + 2-layer MLP head, data-parallel across the NeuronCores
# (batch 256 -> 32 per core, weights/offsets replicated).
#
# The deformable bilinear sampling has offsets shared across batch and
# channels, so each layer's gather+blend is a fixed linear map of the
# input spatial grid. We materialize that map on the host as a dense
# matrix G_l[H*W, K2*Ho*Wo] (4 nonzeros per column) and the device graph
# becomes pure dense matmuls - no gather ops, which the neuron compiler
# handles poorly. Device-side constants (Gs + weights) are cached across
# calls keyed by an input fingerprint so steady-state calls only ship x.

_LAYERS = [(1, 16, 3, 31, 33), (16, 32, 3, 29, 31), (32, 16, 5, 25, 29),
           (16, 16, 7, 19, 25), (16, 8, 5, 15, 19), (8, 4, 3, 13, 15)]
_B = 256
_NC = 8


def _build_G(offset, K, H, W, Ho, Wo):
    """[H*W, K2*Ho*Wo] bilinear sample+blend matrix from fixed offsets."""
    K2 = K * K
    off = np.asarray(offset, np.float64)[0].reshape(K2, 2, Ho, Wo)
    ky, kx = np.meshgrid(np.arange(K), np.arange(K), indexing='ij')
    py = np.arange(Ho)[None, :, None] + ky.reshape(-1, 1, 1) + off[:, 0]
    px = np.arange(Wo)[None, None, :] + kx.reshape(-1, 1, 1) + off[:, 1]
    y0 = np.floor(py).astype(np.int64); x0 = np.floor(px).astype(np.int64)
    wy = (py - y0).astype(np.float32); wx = (px - x0).astype(np.float32)
    G = np.zeros((H * W, K2 * Ho * Wo), np.float32)
    m = np.arange(K2 * Ho * Wo)
    for dy, wyt in ((0, 1.0 - wy), (1, wy)):
        for dx, wxt in ((0, 1.0 - wx), (1, wx)):
            yi = y0 + dy; xi = x0 + dx
            valid = (yi >= 0) & (yi < H) & (xi >= 0) & (xi < W)
            idx = np.clip(yi, 0, H - 1) * W + np.clip(xi, 0, W - 1)
            wt = (wyt * wxt * valid).reshape(-1).astype(np.float32)
            np.add.at(G, (idx.reshape(-1), m), wt)
    return G


def _forward(x, Gs, ws, bs, w7, b7, w8, b8, perm):
    Bn = x.shape[0]
    for (ci, co, K, ho, hi), G, w, b in zip(_LAYERS, Gs, ws, bs):
        K2 = K * K
        s = x.reshape(Bn, ci, hi * hi) @ G            # [B, ci, K2*ho*ho]
        s = s.reshape(Bn, ci, K2, ho * ho)
        out = jnp.einsum('bckp,ock->bop', s, w.reshape(co, ci, K2))
        x = jax.nn.relu(out + b[None, :, None]).reshape(Bn, co, ho, ho)
    x = x.reshape(Bn, 4, 13 * 13)[:, :, perm].reshape(Bn, -1)
    h = jax.nn.relu(x @ w7 + b7)
    return h @ w8 + b8


_cache = {}


def kernel(**inputs):
    x = np.asarray(inputs['x'], np.float32)
    B = x.shape[0]
    n_dev = min(_NC, len(jax.devices()))
    while B % n_dev != 0:
        n_dev //= 2

    h = hashlib.sha1()
    for k in ['off1', 'off2', 'off3', 'off4', 'off5', 'off6',
              'w1', 'b1', 'w2', 'b2', 'w3', 'b3', 'w4', 'b4', 'w5', 'b5',
              'w6', 'b6', 'w7', 'b7', 'w8', 'b8', 'perm']:
        h.update(np.ascontiguousarray(inputs[k]).tobytes())
    key = (h.hexdigest(), n_dev)

    if _cache.get('key') != key:
        Gs = tuple(_build_G(inputs[f'off{i+1}'], K, hi, hi, ho, ho)
                   for i, (ci, co, K, ho, hi) in enumerate(_LAYERS))
        ws = tuple(np.asarray(inputs[f'w{i+1}'], np.float32) for i in range(6))
        bs = tuple(np.asarray(inputs[f'b{i+1}'], np.float32) for i in range(6))
        rest = (np.asarray(inputs['w7'], np.float32),
                np.asarray(inputs['b7'], np.float32),
                np.asarray(inputs['w8'], np.float32),
                np.asarray(inputs['b8'], np.float32),
                np.asarray(inputs['perm']))
        devs = jax.devices()[:n_dev]
        rep = lambda t: jax.device_put_replicated(t, devs)
        _cache['consts'] = jax.tree_util.tree_map(rep, (Gs, ws, bs) + rest)
        _cache['key'] = key
        if 'fn' not in _cache:
            # constants carry a leading replica axis from device_put_replicated,
            # so every pmap arg maps axis 0 and nothing re-transfers per call
            _cache['fn'] = jax.pmap(_forward, in_axes=0, devices=devs)

    xs = x.reshape(n_dev, B // n_dev, *x.shape[1:])
    Gs, ws, bs, w7, b7, w8, b8, perm = _cache['consts']
    out = _cache['fn'](xs, Gs, ws, bs, w7, b7, w8, b8, perm)
    out = np.asarray(out)
    return out.reshape(B, out.shape[-1]).astype(np.float32)



# revision 20
# speedup vs baseline: 1.6279x; 1.6279x over previous
"""nn_ClassifierDeformable on 8 NeuronCores via a hand-written Bass/Tile kernel.

The deformable convs have offsets shared across batch and channels, so each
layer's bilinear gather is a fixed linear map of the input spatial grid. We
materialize it on the host as G_l [Q, K2*P] (4 nnz per column) and run the
whole 6-layer net + MLP head on-chip as dense matmuls:

  per layer (activations stored [q on partitions, (b,c) columns]):
    gather: s_k = act_tile^T @ G_k          (PE, accumulate q-tiles in PSUM)
    mix:    m    = sum_k WBLK_k^T @ s_k      (PE, block-diag weights batch the
                                              32-sample groups over partitions)
    act'   = transpose(relu(m + bias))       (PE transpose back to [p, (b,o)])

Data-parallel over cores: batch 256 -> 32 per core; weights/G replicated.
The compiled NEFF + device-resident weights are cached across calls, so
steady-state calls only ship x and fetch the [256,10] output.
"""

import hashlib
import math
import os

import numpy as np
import ml_dtypes

import concourse.bass as bass
import concourse.mybir as mybir
import concourse.tile as tile

BL = 32          # batch per core
NC = 8           # cores
ACT_DT = os.environ.get("BASS_ACT_DT", "bf16")   # "bf16" | "f32r" | "f32"

# (ci, co, K, ho, hi, Pp, chunks): Pp = padded ho*ho, chunks = (pstart, pc)
# chosen so every chunk is 128-aligned, <=512 wide and (for f32r) >=256.
_L = [
    (1, 16, 3, 31, 33, 1024, ((0, 512), (512, 512))),
    (16, 32, 3, 29, 31, 896, ((0, 512), (512, 384))),
    (32, 16, 5, 25, 29, 640, ((0, 384), (384, 256))),
    (16, 16, 7, 19, 25, 384, ((0, 384),)),
    (16, 8, 5, 15, 19, 256, ((0, 256),)),
    (8, 4, 3, 13, 15, 256, ((0, 256),)),
]


def _np_dt():
    return ml_dtypes.bfloat16 if ACT_DT == "bf16" else np.float32


def _my_dt():
    return {"bf16": mybir.dt.bfloat16, "f32r": mybir.dt.float32r,
            "f32": mybir.dt.float32}[ACT_DT]


# ---------------------------------------------------------------- host prep

def _build_G(offset, K, H, W, Ho, Wo):
    """[H*W, K2*Ho*Wo] bilinear sample+blend matrix from fixed offsets."""
    K2 = K * K
    off = np.asarray(offset, np.float64)[0].reshape(K2, 2, Ho, Wo)
    ky, kx = np.meshgrid(np.arange(K), np.arange(K), indexing="ij")
    py = np.arange(Ho)[None, :, None] + ky.reshape(-1, 1, 1) + off[:, 0]
    px = np.arange(Wo)[None, None, :] + kx.reshape(-1, 1, 1) + off[:, 1]
    y0 = np.floor(py).astype(np.int64)
    x0 = np.floor(px).astype(np.int64)
    wy = (py - y0).astype(np.float32)
    wx = (px - x0).astype(np.float32)
    G = np.zeros((H * W, K2 * Ho * Wo), np.float32)
    m = np.arange(K2 * Ho * Wo)
    for dy, wyt in ((0, 1.0 - wy), (1, wy)):
        for dx, wxt in ((0, 1.0 - wx), (1, wx)):
            yi = y0 + dy
            xi = x0 + dx
            valid = (yi >= 0) & (yi < H) & (xi >= 0) & (xi < W)
            idx = np.clip(yi, 0, H - 1) * W + np.clip(xi, 0, W - 1)
            wt = (wyt * wxt * valid).reshape(-1).astype(np.float32)
            np.add.at(G, (idx.reshape(-1), m), wt)
    return G


def _host_arrays(inputs):
    """All constant tensors, per-core layout, in ACT_DT (except biases noted)."""
    npd = _np_dt()
    out = {}
    for li, (ci, co, K, ho, hi, Pp, _chunks) in enumerate(_L):
        K2, P, Q = K * K, ho * ho, hi * hi
        G = _build_G(inputs[f"off{li+1}"], K, hi, hi, ho, ho)  # [Q, K2*P]
        Qp = ((Q + 127) // 128) * 128
        Gp = np.zeros((Qp, K2 * Pp), np.float32)
        Gp[:Q].reshape(Q, K2, Pp)[:, :, :P] = G.reshape(Q, K2, P)
        out[f"G{li+1}"] = Gp.astype(npd)
        R = min(128, BL * ci)
        g = R // ci
        wb = np.zeros((K2, R, g * co), np.float32)
        wr = np.asarray(inputs[f"w{li+1}"], np.float32).reshape(co, ci, K2)
        for k in range(K2):
            for bg in range(g):
                wb[k, bg * ci:(bg + 1) * ci, bg * co:(bg + 1) * co] = wr[:, :, k].T
        out[f"WB{li+1}"] = wb.astype(npd)
        bias = np.asarray(inputs[f"b{li+1}"], np.float32)
        out[f"BC{li+1}"] = np.tile(bias, g)[None, :].astype(npd)  # [1, g*co]
    # MLP head: fold the spatial permutation into w7.
    perm = np.asarray(inputs["perm"]).astype(np.int64)
    w7 = np.asarray(inputs["w7"], np.float32).reshape(4, 169, 256)
    w7e = np.zeros_like(w7)
    w7e[:, perm, :] = w7                     # w7e[c, q, n]
    w7s = np.zeros((2, 128, 4 * 256), np.float32)  # [qt, qrow, c*256+n]
    for qt in range(2):
        rows = min(128, 169 - qt * 128)
        for c in range(4):
            w7s[qt, :rows, c * 256:(c + 1) * 256] = \
                w7e[c, qt * 128:qt * 128 + rows, :]
    out["W7S"] = w7s.astype(npd)
    out["B7C"] = np.asarray(inputs["b7"], np.float32)[None, :].astype(npd)
    w8 = np.asarray(inputs["w8"], np.float32)       # [256, 10]
    w8s = np.zeros((2, 128, 10), np.float32)
    w8s[0] = w8[:128]
    w8s[1] = w8[128:]
    out["W8S"] = w8s.astype(npd)
    out["B8"] = np.asarray(inputs["b8"], np.float32)[None, :].astype(npd)
    return out


# ---------------------------------------------------------------- bass kernel

def _build_nc():
    from concourse import bacc
    dt = _my_dt()
    f32 = mybir.dt.float32
    nc = bacc.Bacc(None, target_bir_lowering=False)
    io = {}
    io["XT"] = nc.declare_dram_parameter("XT", [1152, BL], dt, isOutput=False)
    for li, (ci, co, K, ho, hi, Pp, _ch) in enumerate(_L):
        K2, Q = K * K, hi * hi
        Qp = ((Q + 127) // 128) * 128
        R = min(128, BL * ci)
        g = R // ci
        io[f"G{li+1}"] = nc.declare_dram_parameter(
            f"G{li+1}", [Qp, K2 * Pp], dt, isOutput=False)
        io[f"WB{li+1}"] = nc.declare_dram_parameter(
            f"WB{li+1}", [K2, R, g * co], dt, isOutput=False)
        io[f"BC{li+1}"] = nc.declare_dram_parameter(
            f"BC{li+1}", [1, g * co], dt, isOutput=False)
    io["W7S"] = nc.declare_dram_parameter("W7S", [2, 128, 1024], dt, isOutput=False)
    io["B7C"] = nc.declare_dram_parameter("B7C", [1, 256], dt, isOutput=False)
    io["W8S"] = nc.declare_dram_parameter("W8S", [2, 128, 10], dt, isOutput=False)
    io["B8"] = nc.declare_dram_parameter("B8", [1, 10], dt, isOutput=False)
    io["OUT"] = nc.declare_dram_parameter("OUT", [BL, 10], f32, isOutput=True)

    with tile.TileContext(nc) as tc:
        _emit(nc, tc, io, dt, f32)
    nc.finalize()
    return nc


def _emit(nc, tc, io, dt, f32):
    from contextlib import ExitStack
    ctx = ExitStack()
    with ctx:
        consts = ctx.enter_context(tc.tile_pool(name="consts", bufs=1))
        acts = ctx.enter_context(tc.tile_pool(name="acts", bufs=1))
        gqp = ctx.enter_context(tc.tile_pool(name="gq", bufs=2))
        wbp = ctx.enter_context(tc.tile_pool(name="wb", bufs=3))
        sp = ctx.enter_context(tc.tile_pool(name="s", bufs=3))
        mp = ctx.enter_context(tc.tile_pool(name="m", bufs=3))
        maccp = ctx.enter_context(tc.tile_pool(name="macc", bufs=8))
        # PSUM budget (8 banks): "pm" 4 + "psg" 2 + "pscr" 2 (pm1/pt shared)
        pg = ctx.enter_context(tc.tile_pool(name="pg", bufs=2, space="PSUM"))
        pm = ctx.enter_context(tc.tile_pool(name="pm", bufs=4, space="PSUM"))
        pscr = ctx.enter_context(tc.tile_pool(name="pscr", bufs=2, space="PSUM"))

        ident = consts.tile([128, 128], dt)
        from concourse.masks import make_identity
        make_identity(nc, ident)
        ones = consts.tile([1, 512], dt)
        nc.gpsimd.memset(ones, 1.0)
        bc_t = []
        for li, (ci, co, K, ho, hi, Pp, _ch) in enumerate(_L):
            g = min(128, BL * ci) // ci
            t = consts.tile([1, g * co], dt, name=f"bc{li+1}")
            nc.sync.dma_start(t, io[f"BC{li+1}"][:, :])
            bc_t.append(t)
        w7s_t = consts.tile([128, 2 * 1024], dt)
        for qt in range(2):
            nc.sync.dma_start(w7s_t[:, qt * 1024:(qt + 1) * 1024], io["W7S"][qt])
        b7c_t = consts.tile([1, 256], dt)
        nc.sync.dma_start(b7c_t, io["B7C"][:, :])
        w8s_t = consts.tile([128, 20], dt)
        for qt in range(2):
            nc.sync.dma_start(w8s_t[:, qt * 10:(qt + 1) * 10], io["W8S"][qt])
        b8_t = consts.tile([1, 10], dt)
        nc.sync.dma_start(b8_t, io["B8"][:, :])

        # activation tiles: [128, nq_tiles * ncols], col block qt holds rows
        # qt*128..qt*128+127 of the [rows, (b,c)] logical activation
        act_rows = [1089] + [lay[5] for lay in _L]        # rows of act_l
        act_cols = [BL * 1] + [BL * lay[1] for lay in _L]  # (b,c) col count
        act_t = []
        for li in range(7):
            nqa = (act_rows[li] + 127) // 128
            t = acts.tile([128, nqa * act_cols[li]], dt, name=f"act{li+1}")
            act_t.append(t)

        # load x (one 3D-AP DMA: [1152, BL] -> [128, (qt, b)])
        nc.sync.dma_start(
            act_t[0].rearrange("p (t c) -> p t c", t=9),
            io["XT"][:, :].rearrange("(t p) c -> p t c", p=128))

        evac_flip = [0]

        def evac(dst, src):  # alternate ACT/DVE for plain psum->sbuf copies
            evac_flip[0] ^= 1
            if evac_flip[0]:
                nc.scalar.activation(dst, src, mybir.ActivationFunctionType.Copy)
            else:
                nc.vector.tensor_copy(dst, src)

        for li, (ci, co, K, ho, hi, Pp, chunks) in enumerate(_L):
            K2, P, Q = K * K, ho * ho, hi * hi
            R = min(128, BL * ci)
            g = R // ci
            gco = g * co
            nt = (BL * ci) // R
            nblk = (gco + 127) // 128
            nqt = (Q + 127) // 128
            ncols = BL * ci
            ain, aout = act_t[li], act_t[li + 1]
            in_psum = nt * nblk <= 4   # accumulate mix over k directly in PSUM

            for (pstart, pc) in chunks:
                if in_psum:
                    pms = [[pm.tile([min(128, gco - blk * 128), pc], f32,
                                    name=f"pm{li}", tag="pm", bufs=4)
                            for blk in range(nblk)] for t in range(nt)]
                    for t in range(nt):
                        for blk in range(nblk):
                            rblk = min(128, gco - blk * 128)
                            nc.tensor.matmul(
                                pms[t][blk], bc_t[li][:, blk * 128:blk * 128 + rblk],
                                ones[:, :pc], start=True, stop=False)
                else:
                    maccs = [[maccp.tile([min(128, gco - blk * 128), pc], f32,
                                         name=f"macc{li}", tag="macc", bufs=8)
                              for blk in range(nblk)] for t in range(nt)]

                for k in range(K2):
                    gq = gqp.tile([128, nqt * pc], dt, name=f"gq{li}",
                                  tag="gq")
                    nc.gpsimd.dma_start(
                        gq.rearrange("p (t c) -> p t c", t=nqt),
                        io[f"G{li+1}"][:, :].rearrange(
                            "(t p) c -> p t c", p=128)[
                            :, :, k * Pp + pstart:k * Pp + pstart + pc])
                    wb = wbp.tile([R, gco], dt, name=f"wb{li}", tag="wb")
                    nc.gpsimd.dma_start(wb, io[f"WB{li+1}"][k])

                    for t in range(nt):
                        psg = pg.tile([R, pc], f32, name="psg", tag="psg")
                        for qt in range(nqt):
                            rows = min(128, Q - qt * 128)
                            nc.tensor.matmul(
                                psg,
                                ain[:rows, qt * ncols + t * R: qt * ncols + t * R + R],
                                gq[:rows, qt * pc:qt * pc + pc],
                                start=(qt == 0), stop=(qt == nqt - 1))
                        s_sb = sp.tile([R, pc], dt, name="s_sb", tag="s_sb")
                        evac(s_sb, psg)
                        for blk in range(nblk):
                            rblk = min(128, gco - blk * 128)
                            if in_psum:
                                nc.tensor.matmul(
                                    pms[t][blk],
                                    wb[:, blk * 128:blk * 128 + rblk],
                                    s_sb, start=False, stop=(k == K2 - 1))
                            else:
                                p1 = pscr.tile([rblk, pc], f32, name="pm1",
                                               tag="pscr")
                                if k == 0:
                                    nc.tensor.matmul(
                                        p1, bc_t[li][:, blk * 128:blk * 128 + rblk],
                                        ones[:, :pc], start=True, stop=False)
                                    nc.tensor.matmul(
                                        p1, wb[:, blk * 128:blk * 128 + rblk],
                                        s_sb, start=False, stop=True)
                                    nc.vector.tensor_copy(maccs[t][blk], p1)
                                else:
                                    nc.tensor.matmul(
                                        p1, wb[:, blk * 128:blk * 128 + rblk],
                                        s_sb, start=True, stop=True)
                                    nc.vector.tensor_add(
                                        maccs[t][blk], maccs[t][blk], p1)

                # relu + transpose back to [p, (b, o)] layout
                for t in range(nt):
                    for blk in range(nblk):
                        rblk = min(128, gco - blk * 128)
                        m_sb = mp.tile([rblk, pc], dt, name="m_sb", tag="m_sb")
                        nc.scalar.activation(
                            m_sb, pms[t][blk] if in_psum else maccs[t][blk],
                            mybir.ActivationFunctionType.Relu)
                        for ps in range(pc // 128):
                            ptile = pscr.tile([128, rblk], dt, name="ptile",
                                              tag="pscr")
                            nc.tensor.transpose(
                                ptile, m_sb[:, ps * 128:(ps + 1) * 128],
                                ident[:rblk, :rblk])
                            qt2 = (pstart + ps * 128) // 128
                            col0 = qt2 * (BL * co) + t * gco + blk * 128
                            evac(aout[:, col0:col0 + rblk], ptile)

        # ---- MLP head ----
        a7 = act_t[6]  # [128, 2*128], col = qt*128 + b*4 + c, rows q of 169
        ps7 = []
        for nb in range(2):
            p7 = pm.tile([128, BL], f32, name="p7", tag="pm", bufs=4)
            nc.tensor.matmul(p7, b7c_t[:, nb * 128:(nb + 1) * 128],
                             ones[:, :BL], start=True, stop=False)
            for c in range(4):
                for qt in range(2):
                    rows = min(128, 169 - qt * 128)
                    nc.tensor.matmul(
                        p7,
                        w7s_t[:rows, qt * 1024 + c * 256 + nb * 128:
                              qt * 1024 + c * 256 + nb * 128 + 128],
                        a7[:rows, qt * 128 + c: qt * 128 + 128: 4],
                        start=False, stop=(c == 3 and qt == 1))
            ps7.append(p7)
        h_sb = consts.tile([128, 2 * BL], dt)
        for nb in range(2):
            nc.scalar.activation(h_sb[:, nb * BL:(nb + 1) * BL], ps7[nb],
                                 mybir.ActivationFunctionType.Relu)
        po = pm.tile([BL, 10], f32, name="po", tag="pm", bufs=4)
        nc.tensor.matmul(po, ones[:, :BL], b8_t[:, :], start=True, stop=False)
        for nb in range(2):
            nc.tensor.matmul(po, h_sb[:, nb * BL:(nb + 1) * BL],
                             w8s_t[:, nb * 10:(nb + 1) * 10],
                             start=False, stop=(nb == 1))
        out_sb = consts.tile([BL, 10], f32)
        nc.scalar.activation(out_sb, po, mybir.ActivationFunctionType.Copy)
        nc.sync.dma_start(io["OUT"][:, :], out_sb)


# ---------------------------------------------------------------- runner

_STATE = {}


def _make_runner(nc, n_cores):
    import jax
    from jax.sharding import Mesh, PartitionSpec, NamedSharding
    from jax.experimental.shard_map import shard_map
    from concourse.bass2jax import (
        _bass_exec_p, install_neuronx_cc_hook, partition_id_tensor)

    install_neuronx_cc_hook()

    partition_name = (nc.partition_id_tensor.name
                      if nc.partition_id_tensor else None)
    in_names, out_names, out_avals, zero_outs = [], [], [], []
    for alloc in nc.m.functions[0].allocations:
        if not isinstance(alloc, mybir.MemoryLocationSet):
            continue
        name = alloc.memorylocations[0].name
        if alloc.kind == "ExternalInput":
            if name != partition_name:
                in_names.append(name)
        elif alloc.kind == "ExternalOutput":
            out_names.append(name)
            shape = tuple(alloc.tensor_shape)
            dtype = mybir.dt.np(alloc.dtype)
            out_avals.append(jax.core.ShapedArray(shape, dtype))
            zero_outs.append((shape, dtype))
    n_params = len(in_names)
    all_names = in_names + out_names
    if partition_name is not None:
        all_names = all_names + [partition_name]

    def _body(*args):
        operands = list(args)
        if partition_name is not None:
            operands.append(partition_id_tensor())
        outs = _bass_exec_p.bind(
            *operands,
            out_avals=tuple(out_avals),
            in_names=tuple(all_names),
            out_names=tuple(out_names),
            lowering_input_output_aliases=(),
            sim_require_finite=True,
            sim_require_nnan=True,
            nc=nc,
        )
        return tuple(outs)

    devices = jax.devices()[:n_cores]
    mesh = Mesh(np.asarray(devices), ("core",))
    n_out = len(out_names)
    donate = tuple(range(n_params, n_params + n_out))
    sharded = jax.jit(
        shard_map(_body, mesh=mesh,
                  in_specs=(PartitionSpec("core"),) * (n_params + n_out),
                  out_specs=(PartitionSpec("core"),) * n_out,
                  check_rep=False),
        donate_argnums=donate, keep_unused=True)
    sharding = NamedSharding(mesh, PartitionSpec("core"))
    return sharded, in_names, out_names, zero_outs, sharding


def _fingerprint(inputs):
    h = hashlib.sha1()
    for k in sorted(inputs.keys()):
        if k == "x":
            continue
        h.update(k.encode())
        h.update(np.ascontiguousarray(inputs[k]).tobytes())
    return h.hexdigest()


def kernel(**inputs):
    import jax
    x = np.asarray(inputs["x"], np.float32)
    B = x.shape[0]
    assert B == NC * BL, f"expected batch {NC * BL}, got {B}"
    key = (_fingerprint(inputs), ACT_DT)

    if _STATE.get("key") != key:
        host = _host_arrays(inputs)
        if "runner" not in _STATE:
            nc = _build_nc()
            _STATE["runner"] = _make_runner(nc, NC)
        sharded, in_names, out_names, zero_outs, sharding = _STATE["runner"]
        consts = {}
        for name, arr in host.items():
            glob = np.concatenate([arr] * NC, axis=0)
            consts[name] = jax.device_put(glob, sharding)
        _STATE["consts"] = consts
        _STATE["key"] = key

    sharded, in_names, out_names, zero_outs, sharding = _STATE["runner"]
    npd = _np_dt()
    xt_full = np.zeros((NC, 1152, BL), np.float32)
    for c in range(NC):
        xt_full[c, :1089] = x[c * BL:(c + 1) * BL].reshape(BL, 1089).T
    xt = xt_full.reshape(NC * 1152, BL).astype(npd)
    args = []
    for name in in_names:
        args.append(xt if name == "XT" else _STATE["consts"][name])
    for shape, dtype in zero_outs:
        args.append(np.zeros((NC * shape[0],) + shape[1:], dtype))
    outs = sharded(*args)
    out = np.asarray(outs[0])  # [NC*BL, 10]
    return out.astype(np.float32)
